# revision 2
# baseline (speedup 1.0000x reference)
"""ClusterGCN layer on 8 Trainium2 NeuronCores.

Strategy: shard nodes by cluster (greedy balance) so every intra-cluster
edge is device-local.  The key algebraic move: aggregation commutes with
the linear transform,

    out = a*(X@W) + (A_norm X)@W + bias,   a = dinv^2 (self loop)

so the sparse aggregate runs on the *input* features and has no
dependency on computed outputs.  Per device:

  - dense phase (W-stationary): psum[m, cols] += W[k,m].T @ xt[k, cols]
    over k, producing out.T directly; PSUM evicted with a fused
    per-partition bias add + f32->bf16 cast, split across DVE and ACT.
  - sparse phase: edges sorted by dst are packed into 128-slot blocks;
    Y.T_k = G_k.T @ S per window (G = packed src rows, S = norm-valued
    one-hot), landing in the transposed layout the dense stream wants.
    Window w's dst nodes occupy local columns [w*128,(w+1)*128), so the
    dense matmul simply accumulates extra passes over yt for the low
    columns - no gather, no scatter, no gpsimd.

All tensors use large (>=2KB) DMA descriptors via host-side packing;
output returns bf16 and is unscrambled + upcast on the host.
"""

import numpy as np

N = 100000
D = 256
C = 64
M = 8  # cores

_SB = 1024  # dense superblock (node columns per psum tile)


def _build_program(NCAP, NWCAP, BPW):
    import concourse.bacc as bacc
    import concourse.mybir as mybir
    from concourse import tile

    f32 = mybir.dt.float32
    bf16 = mybir.dt.bfloat16
    NBLK = max(int(sum(BPW)), 1)
    YC = NWCAP * 128  # yt columns (dst windows)

    nc = bacc.Bacc("TRN2", target_bir_lowering=False, debug=False, num_devices=M)

    XTD = nc.dram_tensor("xtd", [128, 2, NCAP], bf16, kind="ExternalInput")
    WD = nc.dram_tensor("wd", [128, 2, 2, 128], bf16, kind="ExternalInput")
    BD = nc.dram_tensor("bd", [128, 2], f32, kind="ExternalInput")
    GD = nc.dram_tensor("gd", [128, NBLK, 2, 128], bf16, kind="ExternalInput")
    SD = nc.dram_tensor("sd", [128, NBLK, 128], bf16, kind="ExternalInput")
    OUTD = nc.dram_tensor("outd", [128, 2, NCAP], bf16, kind="ExternalOutput")

    # block offsets per window
    b0 = np.concatenate([[0], np.cumsum(BPW)]).astype(int) if NWCAP else np.array([0])

    # superblocks: full _SB plus optional 512 tail; yt-overlapping ones last
    sbs = []
    c0 = 0
    while c0 < NCAP:
        ln = min(_SB, NCAP - c0)
        sbs.append((c0, ln))
        c0 += ln
    n_low = sum(1 for s0, ln in sbs if s0 < YC)
    order = list(range(n_low, len(sbs))) + list(range(n_low))

    with tile.TileContext(nc) as tc:
        with (
            tc.tile_pool(name="const", bufs=1) as cpool,
            tc.tile_pool(name="dps", bufs=3, space="PSUM") as dpool,
            tc.tile_pool(name="sps", bufs=2, space="PSUM") as spool,
        ):
            xt = cpool.tile([128, 2, NCAP], bf16)
            ot = cpool.tile([128, 2, NCAP], bf16)
            gt = cpool.tile([128, NBLK, 2, 128], bf16)
            st = cpool.tile([128, NBLK, 128], bf16)
            wt = cpool.tile([128, 2, 2, 128], bf16)
            bt = cpool.tile([128, 2], f32)
            yt = cpool.tile([128, 2, max(YC, 128)], bf16)

            # ---- input DMAs (issue order = queue order) ----
            nc.sync.dma_start(wt[:], WD[:])
            nc.sync.dma_start(bt[:], BD[:])
            # first two superblocks of the high (non-yt) range
            pre = []
            for i in order[:2]:
                s0, ln = sbs[i]
                nc.sync.dma_start(xt[:, :, s0 : s0 + ln], XTD[:, :, s0 : s0 + ln])
                pre.append(i)
            if NWCAP:
                nc.sync.dma_start(gt[:], GD[:])
                nc.sync.dma_start(st[:], SD[:])
            # rest of xt in processing order, 2 superblocks per DMA
            rest = [i for i in order if i not in pre]
            j = 0
            while j < len(rest):
                grp = [rest[j]]
                if j + 1 < len(rest) and rest[j + 1] == rest[j] + 1:
                    grp.append(rest[j + 1])
                    j += 2
                else:
                    j += 1
                s0 = sbs[grp[0]][0]
                s1 = sbs[grp[-1]][0] + sbs[grp[-1]][1]
                nc.sync.dma_start(xt[:, :, s0:s1], XTD[:, :, s0:s1])

            mm = nc.tensor.matmul
            ev_flip = [0]

            def dense_sb(i):
                s0, ln = sbs[i]
                for m in range(2):
                    ps = dpool.tile([128, _SB], f32, tag="d")
                    # per-512-region contribution lists for start/stop flags
                    nreg = (ln + 511) // 512
                    regs = []
                    for r in range(nreg):
                        c = s0 + r * 512
                        w = min(512, s0 + ln - c)
                        n_c = 2  # xt contributions (k=0,1)
                        if c < YC:
                            n_c += 2  # yt contributions
                        regs.append([c, w, n_c, 0])
                    for k in range(2):
                        for r in range(nreg):
                            c, w, n_c, done = regs[r]
                            mm(
                                ps[:, r * 512 : r * 512 + w],
                                wt[:, k, m, :],
                                xt[:, k, c : c + w],
                                start=(done == 0),
                                stop=(done == n_c - 1),
                            )
                            regs[r][3] += 1
                        for r in range(nreg):
                            c, w, n_c, done = regs[r]
                            if c >= YC:
                                continue
                            yw = min(w, YC - c)
                            mm(
                                ps[:, r * 512 : r * 512 + yw],
                                wt[:, k, m, :],
                                yt[:, k, c : c + yw],
                                start=False,
                                stop=(done == n_c - 1),
                            )
                            regs[r][3] += 1
                    dst = ot[:, m, s0 : s0 + ln]
                    if ev_flip[0] % 2 == 0:
                        nc.vector.tensor_scalar_add(dst, ps[:, :ln], bt[:, m : m + 1])
                    else:
                        nc.scalar.add(dst, ps[:, :ln], bt[:, m : m + 1])
                    ev_flip[0] += 1
                nc.sync.dma_start(OUTD[:, :, s0 : s0 + ln], ot[:, :, s0 : s0 + ln])

            def sparse_phase():
                # windows in groups of 4 -> one [128,512] psum per (k, group)
                cp_flip = 0
                for k in range(2):
                    w = 0
                    while w < NWCAP:
                        g = min(4, NWCAP - w)
                        ps = spool.tile([128, 512], f32, tag="s")
                        for wi in range(g):
                            nb = BPW[w + wi]
                            for b in range(nb):
                                blk = b0[w + wi] + b
                                mm(
                                    ps[:, wi * 128 : (wi + 1) * 128],
                                    gt[:, blk, k, :],
                                    st[:, blk, :],
                                    start=(b == 0),
                                    stop=(b == nb - 1),
                                )
                        dst = yt[:, k, w * 128 : (w + g) * 128]
                        if cp_flip % 2 == 0:
                            nc.scalar.copy(dst, ps[:, : g * 128])
                        else:
                            nc.vector.tensor_scalar_add(dst, ps[:, : g * 128], 0.0)
                        cp_flip += 1
                        w += g

            # emission order: 2 dense superblocks, sparse phase, rest, low last
            for i in order[:2]:
                dense_sb(i)
            if NWCAP:
                sparse_phase()
            for i in order[2:]:
                dense_sb(i)

    nc.compile()
    return nc


def _run_program(nc, in_maps):
    from concourse.bass_utils import run_bass_kernel_spmd

    return run_bass_kernel_spmd(nc, in_maps, core_ids=list(range(M))).results


def _ceil_to(x, m):
    return -(-x // m) * m


def kernel(X, weight, bias, cluster_assignment, edge_index):
    import ml_dtypes

    bf = ml_dtypes.bfloat16
    X = np.ascontiguousarray(np.asarray(X, dtype=np.float32))
    weight = np.ascontiguousarray(np.asarray(weight, dtype=np.float32))
    bias = np.asarray(bias, dtype=np.float32)
    cl = np.asarray(cluster_assignment).astype(np.int64)
    ei = np.asarray(edge_index).astype(np.int64)

    src, dst = ei[0], ei[1]
    intra = cl[src] == cl[dst]
    es, ed = src[intra], dst[intra]

    deg = (np.bincount(ed, minlength=N) + 1.0).astype(np.float32)
    dinv = (1.0 / np.sqrt(deg)).astype(np.float32)

    # clusters -> devices, greedy balance by node count
    csize = np.bincount(cl, minlength=C)
    devn = np.zeros(M, dtype=np.int64)
    cdev = np.zeros(C, dtype=np.int64)
    for c in np.argsort(-csize, kind="stable"):
        d = int(np.argmin(devn))
        cdev[c] = d
        devn[d] += csize[c]
    node_dev = cdev[cl]
    edge_dev = node_dev[ed]

    # per-device layouts
    devs = []
    for d in range(M):
        nodes_d = np.where(node_dev == d)[0]
        em = edge_dev == d
        esd, edd = es[em], ed[em]
        nrm = (dinv[esd] * dinv[edd]).astype(np.float32)
        o = np.argsort(edd, kind="stable")
        esd, edd, nrm = esd[o], edd[o], nrm[o]
        udst, degs = (
            np.unique(edd, return_counts=True) if edd.size else (edd[:0], edd[:0])
        )
        # greedy windows: <=128 dsts, close when edges would exceed 128
        # (a single dst with >128 edges gets its own multi-block window)
        wins = []  # (ndst, nedge)
        cur_d = cur_e = 0
        for g in degs:
            g = int(g)
            if cur_d and (cur_d >= 128 or (cur_e + g > 128 and cur_e > 0)):
                wins.append((cur_d, cur_e))
                cur_d = cur_e = 0
            cur_d += 1
            cur_e += g
        if cur_d:
            wins.append((cur_d, cur_e))
        devs.append(
            dict(nodes_d=nodes_d, esd=esd, edd=edd, nrm=nrm, udst=udst,
                 degs=degs, wins=wins)
        )

    NWCAP = max(len(dv["wins"]) for dv in devs)
    max_nd = max(dv["nodes_d"].size for dv in devs)
    NCAP = _ceil_to(max_nd, 512)
    assert NWCAP * 128 <= NCAP
    BPW = np.zeros(NWCAP, dtype=np.int64)
    for dv in devs:
        for w, (nd_w, ne_w) in enumerate(dv["wins"]):
            BPW[w] = max(BPW[w], -(-ne_w // 128))
    NBLK = max(int(BPW.sum()), 1)
    b0 = np.concatenate([[0], np.cumsum(BPW)]).astype(int)

    w_pack = np.ascontiguousarray(
        weight.reshape(2, 128, 2, 128).transpose(1, 0, 2, 3).astype(bf)
    )
    b_pack = np.ascontiguousarray(bias.reshape(2, 128).T)

    in_maps = []
    for dv in devs:
        nodes_d, udst, degs, wins = dv["nodes_d"], dv["udst"], dv["degs"], dv["wins"]
        nd = nodes_d.size
        # local order: window dsts (+ filler non-dst nodes to 128 per window),
        # then the remaining nodes
        is_dst = np.zeros(N, dtype=bool)
        is_dst[udst] = True
        others = nodes_d[~is_dst[nodes_d]]
        lo = np.empty(nd, dtype=np.int64)
        oi = di = 0
        pos = 0
        for nd_w, ne_w in wins:
            lo[pos : pos + nd_w] = udst[di : di + nd_w]
            di += nd_w
            nf = 128 - nd_w
            lo[pos + nd_w : pos + 128] = others[oi : oi + nf]
            oi += nf
            pos += 128
        lo[pos:] = others[oi:]

        # xt: a*X rows, transposed, padded
        a = dinv[lo] * dinv[lo]
        arr = np.zeros((NCAP, D), dtype=np.float32)
        arr[:nd] = X[lo] * a[:, None]
        xtd = np.ascontiguousarray(arr.reshape(NCAP, 2, 128).transpose(2, 1, 0).astype(bf))

        # G (packed src rows) and S (norm one-hot) per block
        g_rows = np.zeros((NBLK * 128, D), dtype=np.float32)
        s_mat = np.zeros((NBLK * 128, 128), dtype=np.float32)
        e0 = d0 = 0
        for w, (nd_w, ne_w) in enumerate(wins):
            sl0 = b0[w] * 128
            g_rows[sl0 : sl0 + ne_w] = X[dv["esd"][e0 : e0 + ne_w]]
            cols = np.repeat(np.arange(nd_w), degs[d0 : d0 + nd_w])
            s_mat[sl0 + np.arange(ne_w), cols] = dv["nrm"][e0 : e0 + ne_w]
            e0 += ne_w
            d0 += nd_w
        gd = np.ascontiguousarray(
            g_rows.reshape(NBLK, 128, 2, 128).transpose(1, 0, 2, 3).astype(bf)
        )
        sd = np.ascontiguousarray(
            s_mat.reshape(NBLK, 128, 128).transpose(1, 0, 2).astype(bf)
        )
        in_maps.append(
            {"xtd": xtd, "wd": w_pack, "bd": b_pack, "gd": gd, "sd": sd}
        )
        dv["lo"] = lo

    nc = _build_program(NCAP, NWCAP, [int(x) for x in BPW])
    results = _run_program(nc, in_maps)

    out = np.empty((N, D), dtype=np.float32)
    for d, dv in enumerate(devs):
        nd = dv["nodes_d"].size
        rows = results[d]["outd"].transpose(2, 1, 0).reshape(NCAP, D)[:nd]
        out[dv["lo"]] = rows.astype(np.float32)

    # clusters with no intra edges keep X
    epc = np.bincount(cl[ed], minlength=C)
    inactive = np.where(epc[cl] == 0)[0]
    if inactive.size:
        out[inactive] = X[inactive]
    return out
